# revision 30
# baseline (speedup 1.0000x reference)
"""CrossModalFusion Trainium2 kernel (v5).

Reference computation (per batch b):
    q = rgb @ Wq + bq                 [S, H]
    k = pose @ Wk + bk                [S, H]
    v = pose @ Wv + bv                [S, H]
    attn = softmax(q @ k.T / sqrt(H)) [S, S]
    out  = attn @ v                   [S, H]
    proj = out @ Wp + bp              [S, D]
    x = rgb + gate * proj
    fused = LayerNorm(x) * gamma + beta

Sharding: pure data-parallel over batch B=32 across 8 NeuronCores
(4 batches per core), identical SPMD program, no collectives.

Design notes:
  - Host pre-processing: rgb/pose cast to bf16, zero-padded d 400->512
    (full-128 contraction chunks keep the PE's HAM activity monitor at
    the 2.4 GHz un-throttled clock) and PRE-TRANSPOSED to
    [b, 4, 128, s] so the kernel only does contiguous DMA loads --
    zero transpose instructions on the device.
  - QKV biases are folded into the matmuls: padded input column 400
    is set to 1.0 and row 400 of the padded Wq/Wk/Wv carries the
    bias, so PSUM drains are plain dtype-converting copies.
  - QKV/proj matmuls in bf16, q/k/v/attn stored fp8e4; the two big
    attention matmuls + column sums run fp8 DoubleRow (256-deep
    contraction per instruction, ~2x bf16 FLOP rate).
  - exp computed as exp(s/sqrt(H) - SHIFT) so unnormalized weights fit
    fp8e4's +-240 range (TRN e4m3 NaNs above 240); the shift cancels
    in the softmax normalization.
  - ACT runs only Exp + PSUM copies (no activation-table reloads);
    LayerNorm rsqrt is a bit-trick+Newton chain on DVE, batched over
    each query block's four row tiles.
  - Next batch's kT matmuls are emitted before the last query block's
    epilogue (and v right after it) so the PE never idles across batch
    boundaries.
"""

import numpy as np

B, S, D, H = 32, 2048, 400, 512
DP = 512                 # d padded to a multiple of 128
N_CORES = 8
B_LOC = B // N_CORES
LN_EPS = 1e-5
P = 128                  # partitions
QBLK = 512               # query block (columns of scoresT)
SHIFT = 2.5              # exp(s - SHIFT): keeps fp8 attn weights < 240

WEIGHT_NAMES = ("Wq", "bq", "Wk", "bk", "Wv", "bv", "Wp", "bp",
                "ln_gamma", "ln_beta", "gate")
SHARDED_NAMES = ("poseT", "pose8T", "rgbT", "rgb32")


def prepare_inputs(inputs):
    """Host-side preprocessing: cast the big activations to bf16 (and
    fp8 for the v path), zero-pad d 400->DP with a 1.0 in column d
    (bias row), and pre-transpose to the [b, DP/128, 128, s] layout
    the kernel streams with plain contiguous DMAs. QKV biases ride in
    row d of the padded weights."""
    import ml_dtypes

    bf16 = ml_dtypes.bfloat16
    fp8 = ml_dtypes.float8_e4m3
    rgb = np.asarray(inputs["rgb"], dtype=np.float32)
    pose = np.asarray(inputs["pose"], dtype=np.float32)
    b, s, d = rgb.shape

    def pad_cast_t(x, dt):
        out = np.zeros((b, s, DP), dtype=dt)
        out[:, :, :d] = x.astype(dt)
        out[:, :, d] = 1.0          # bias row
        # [b, s, (c p)] -> [b, c, p, s]
        return np.ascontiguousarray(
            out.reshape(b, s, DP // P, P).transpose(0, 2, 3, 1))

    def pad_w(w, bias, dt):
        out = np.zeros((DP, H), dtype=dt)
        out[:d] = np.asarray(w, dtype=np.float32).astype(dt)
        out[d] = np.asarray(bias, dtype=np.float32).astype(dt)
        return out

    staged = {
        "poseT": pad_cast_t(pose, bf16),
        "pose8T": pad_cast_t(pose, fp8),
        "rgbT": pad_cast_t(rgb, bf16),
        "rgb32": np.ascontiguousarray(rgb),
        "Wq": pad_w(inputs["Wq"], inputs["bq"], bf16),
        "Wk": pad_w(inputs["Wk"], inputs["bk"], bf16),
        "Wv": pad_w(inputs["Wv"], inputs["bv"], fp8),
        # x16 lifts Wp (std ~1/sqrt(H)) out of fp8's subnormal range; it
        # exactly cancels the kernel's 1/16 outT scale.
        "Wp": (np.asarray(inputs["Wp"], dtype=np.float32) * 16.0).astype(fp8),
    }
    for name in ("bp", "ln_gamma", "ln_beta", "gate"):
        staged[name] = np.ascontiguousarray(inputs[name], dtype=np.float32)
    return staged


def build_nc(b_loc=B_LOC, s=S, d=D, h=H):
    import concourse.bass as bass
    import concourse.mybir as mybir
    import concourse.tile as tile
    from concourse import bacc

    def bcast(ap1d, p=P):
        """Broadcast a 1-D DRAM AP across p partitions (step-0 leading dim)."""
        return bass.AP(tensor=ap1d.tensor, offset=ap1d.offset,
                       ap=[[0, p]] + list(ap1d.ap))

    f32 = mybir.dt.float32
    bf16 = mybir.dt.bfloat16
    fp8 = mybir.dt.float8e4
    i32 = mybir.dt.int32
    AF = mybir.ActivationFunctionType
    DR = mybir.MatmulPerfMode.DoubleRow
    OP = mybir.AluOpType

    nt = s // P              # seq tiles (16)
    nqb = s // QBLK          # query blocks (4)
    tpb = QBLK // P          # seq tiles per query block (4)
    nht = h // P             # h chunks (4)
    ndc = DP // P            # padded-d chunks (4)
    scale = 1.0 / float(np.sqrt(h))

    nc = bacc.Bacc("TRN2", target_bir_lowering=False, debug=False,
                   num_swdge_queues=4)

    poseT_d = nc.dram_tensor("poseT", [b_loc, ndc, P, s], bf16,
                             kind="ExternalInput").ap()
    pose8T_d = nc.dram_tensor("pose8T", [b_loc, ndc, P, s], fp8,
                              kind="ExternalInput").ap()
    rgbT_d = nc.dram_tensor("rgbT", [b_loc, ndc, P, s], bf16,
                            kind="ExternalInput").ap()
    rgb32 = nc.dram_tensor("rgb32", [b_loc, s, d], f32,
                           kind="ExternalInput").ap()
    Wq = nc.dram_tensor("Wq", [DP, h], bf16, kind="ExternalInput").ap()
    Wk = nc.dram_tensor("Wk", [DP, h], bf16, kind="ExternalInput").ap()
    Wv = nc.dram_tensor("Wv", [DP, h], fp8, kind="ExternalInput").ap()
    Wp = nc.dram_tensor("Wp", [h, d], fp8, kind="ExternalInput").ap()
    bp = nc.dram_tensor("bp", [d], f32, kind="ExternalInput").ap()
    gamma = nc.dram_tensor("ln_gamma", [d], f32, kind="ExternalInput").ap()
    beta = nc.dram_tensor("ln_beta", [d], f32, kind="ExternalInput").ap()
    gate = nc.dram_tensor("gate", [1], f32, kind="ExternalInput").ap()
    out = nc.dram_tensor("out", [b_loc, s, d], f32, kind="ExternalOutput").ap()

    from contextlib import ExitStack

    with tile.TileContext(nc) as tc:
        with ExitStack() as ctx:
            pool = lambda **kw: ctx.enter_context(tc.tile_pool(**kw))
            const = pool(name="const", bufs=1)
            wpool = pool(name="wpool", bufs=1)
            ptp = pool(name="ptp", bufs=2)            # poseT bf16
            p8tp = pool(name="p8tp", bufs=2)          # poseT fp8
            rtp = pool(name="rtp", bufs=2)            # rgbT bf16
            ktp = pool(name="ktp", bufs=1)            # kT fp8
            vtp = pool(name="vtp", bufs=1)            # v fp8
            qtp = pool(name="qtp", bufs=2)            # qT fp8
            atp = pool(name="atp", bufs=2)            # attnT fp8
            otp = pool(name="otp", bufs=2)            # outT bf16
            rres = pool(name="rres", bufs=2 * tpb)    # rgb32 rows f32
            small = pool(name="small", bufs=6)
            cspool = pool(name="cspool", bufs=2)
            ypool = pool(name="ypool", bufs=2 * tpb)
            ps_mm = pool(name="ps_mm", bufs=4, space="PSUM")
            ps_sc = pool(name="ps_sc", bufs=2, space="PSUM")
            ps_cs = pool(name="ps_cs", bufs=1, space="PSUM")
            ps_r1 = pool(name="ps_r1", bufs=1, space="PSUM")

            # ---- constants ----
            ones8 = const.tile([P, 2, 16], fp8)
            nc.vector.memset(ones8, 1.0)
            ones_1x4 = const.tile([1, 4], f32)
            nc.vector.memset(ones_1x4, 1.0)
            nshift = const.tile([P, 1], f32)
            nc.vector.memset(nshift, -SHIFT)

            # input loads for batch 0 (pose chunks first: phase A only
            # needs poseT, so the PE can start as soon as they land)
            def emit_in_loads(b):
                poseT = ptp.tile([P, ndc, s], bf16, tag="poseT")
                pose8T = p8tp.tile([P, ndc, s], fp8, tag="pose8T")
                rgbT = rtp.tile([P, ndc, s], bf16, tag="rgbT")
                for c in range(ndc):
                    nc.sync.dma_start(out=poseT[:, c, :],
                                      in_=poseT_d[b, c, :, :])
                for c in range(ndc):
                    nc.sync.dma_start(out=pose8T[:, c, :],
                                      in_=pose8T_d[b, c, :, :])
                    nc.sync.dma_start(out=rgbT[:, c, :],
                                      in_=rgbT_d[b, c, :, :])
                return poseT, pose8T, rgbT

            in_state = emit_in_loads(0)

            # HAM warm-up: dense full-array matmuls bridging the initial
            # DMA window so phase A starts at the 2.4 GHz clock. Emitted
            # before the weight loads so the first matmul isn't queued
            # behind DMA-dependent DVE work.
            ident = const.tile([P, P], bf16)
            nc.vector.memset(ident, 0.0)
            warm = ps_mm.tile([P, QBLK], f32, tag="mm")
            for _ in range(140):
                nc.tensor.matmul(warm[:, :P], ident, ident,
                                 start=True, stop=True)

            # weights: Wk first (phase A starts with kT), then Wv, Wq, Wp
            wk_sb = wpool.tile([P, ndc, h], bf16)
            wv_sb = wpool.tile([P, ndc, h], fp8)
            wq_sb = wpool.tile([P, ndc, h], bf16)
            for dst, W in ((wk_sb, Wk), (wv_sb, Wv), (wq_sb, Wq)):
                for c in range(ndc):
                    nc.gpsimd.dma_start(out=dst[:, c, :],
                                        in_=W[c * P:(c + 1) * P, :])
            wp_sb = wpool.tile([P, nht, d], fp8)
            for t in range(nht):
                nc.gpsimd.dma_start(out=wp_sb[:, t, :],
                                    in_=Wp[t * P:(t + 1) * P, :])

            bp_bc = wpool.tile([P, d], f32)
            nc.gpsimd.dma_start(out=bp_bc, in_=bcast(bp))
            gamma_bc = wpool.tile([P, d], f32)
            nc.gpsimd.dma_start(out=gamma_bc, in_=bcast(gamma))
            beta_bc = wpool.tile([P, d], f32)
            nc.gpsimd.dma_start(out=beta_bc, in_=bcast(beta))
            gate_sb = wpool.tile([P, 1], f32)
            nc.gpsimd.dma_start(out=gate_sb, in_=bcast(gate))
            # bpg = gate * bp (added to rgb once per row tile)
            bpg_bc = wpool.tile([P, d], f32)
            nc.vector.tensor_scalar_mul(out=bpg_bc, in0=bp_bc, scalar1=gate_sb)

            def rsqrt_dve(var_eps, gbuf, n):
                """gbuf[:, :n] <- 1/sqrt(var_eps[:, :n]) on DVE.

                Bit-trick seed + 2 Newton iterations (final rel err <1e-5)."""
                gi = gbuf.bitcast(i32)
                nc.vector.tensor_scalar(
                    out=gi, in0=var_eps.bitcast(i32), scalar1=1, scalar2=None,
                    op0=OP.arith_shift_right)
                nc.vector.tensor_scalar(
                    out=gi, in0=gi, scalar1=-1, scalar2=0x5F3759DF,
                    op0=OP.mult, op1=OP.add)
                t = small.tile([P, n], f32, tag="nwt")
                for _ in range(2):
                    nc.vector.tensor_mul(out=t, in0=gbuf, in1=gbuf)
                    nc.vector.tensor_mul(out=t, in0=t, in1=var_eps)
                    nc.vector.tensor_scalar(
                        out=t, in0=t, scalar1=-0.5, scalar2=1.5,
                        op0=OP.mult, op1=OP.add)
                    nc.vector.tensor_mul(out=gbuf, in0=gbuf, in1=t)

            def emit_kT(poseT):
                """kT[h, :] = Wk.T-chunks @ poseT (bias folded in)."""
                kT = ktp.tile([P, nht, s], fp8, tag="kT")
                for ht in range(nht):
                    for nb in range(s // QBLK):
                        ps = ps_mm.tile([P, QBLK], f32, tag="mm")
                        for c in range(ndc):
                            nc.tensor.matmul(
                                ps,
                                wk_sb[:, c, ht * P:(ht + 1) * P],
                                poseT[:, c, nb * QBLK:(nb + 1) * QBLK],
                                start=(c == 0), stop=(c == ndc - 1),
                            )
                        nc.scalar.copy(
                            out=kT[:, ht, nb * QBLK:(nb + 1) * QBLK], in_=ps)
                return kT

            def emit_v(pose8T):
                """v[sk, h] seq-major (bias folded in), fp8 DoubleRow."""
                v_sb = vtp.tile([P, nt, h], fp8, tag="v")
                for t in range(nt):
                    ps = ps_mm.tile([P, h], f32, tag="mm")
                    for cc in range(ndc // 2):
                        nc.tensor.matmul(
                            ps,
                            pose8T[:, 2 * cc:2 * cc + 2, t * P:(t + 1) * P],
                            wv_sb[:, 2 * cc:2 * cc + 2, :],
                            start=(cc == 0), stop=(cc == ndc // 2 - 1),
                            perf_mode=DR,
                        )
                    nc.vector.tensor_copy(out=v_sb[:, t, :], in_=ps)
                return v_sb

            def emit_qt(b, qb, rgbT):
                """qT block (bf16 matmul, bias folded) + residual rows.

                Emitted one query block ahead so the qT drains clear the
                engine queues before scores need them."""
                q0 = qb * QBLK
                qT = qtp.tile([P, nht, QBLK], fp8, tag="qT")
                rgb_raw = []
                for j in range(tpb):
                    rr = rres.tile([P, d], f32, tag="rr")
                    nc.sync.dma_start(
                        out=rr, in_=rgb32[b, q0 + j * P:q0 + (j + 1) * P, :])
                    rgb_raw.append(rr)
                for ht in range(nht):
                    ps = ps_mm.tile([P, QBLK], f32, tag="mm")
                    for c in range(ndc):
                        nc.tensor.matmul(
                            ps,
                            wq_sb[:, c, ht * P:(ht + 1) * P],
                            rgbT[:, c, q0:q0 + QBLK],
                            start=(c == 0), stop=(c == ndc - 1),
                        )
                    nc.scalar.copy(out=qT[:, ht, :], in_=ps)
                # residual base: rgb += gate*bp
                for j in range(tpb):
                    nc.vector.tensor_add(
                        out=rgb_raw[j], in0=rgb_raw[j], in1=bpg_bc)
                return qT, rgb_raw

            kT, v_sb = emit_kT(in_state[0]), emit_v(in_state[1])
            qstate = emit_qt(0, 0, in_state[2])

            for b in range(b_loc):
                poseT, pose8T, rgbT = in_state
                next_in = next_kT = next_v = None
                if b + 1 < b_loc:
                    in_state = emit_in_loads(b + 1)
                    next_in = in_state

                for qb in range(nqb):
                    q0 = qb * QBLK
                    qT, rgb_raw = qstate

                    # scoresT tiles [sk 128, sq QBLK] via fp8 DoubleRow;
                    # exp(s*scale - SHIFT) -> attnT (fp8)
                    attnT = atp.tile([P, nt, QBLK], fp8, tag="attnT")
                    for c in range(nt):
                        ps = ps_sc.tile([P, QBLK], f32, tag="sc")
                        for jj in range(nht // 2):
                            nc.tensor.matmul(
                                ps,
                                kT[:, 2 * jj:2 * jj + 2, c * P:(c + 1) * P],
                                qT[:, 2 * jj:2 * jj + 2, :],
                                start=(jj == 0), stop=(jj == nht // 2 - 1),
                                perf_mode=DR,
                            )
                        nc.scalar.activation(
                            out=attnT[:, c, :], in_=ps, func=AF.Exp,
                            scale=scale, bias=nshift)

                    # outT[h, sq] = sum_t v[t].T-pair @ attnT[t-pair] (DR)
                    outT = otp.tile([P, nht, QBLK], fp8, tag="outT")
                    for ht in range(nht):
                        ps = ps_mm.tile([P, QBLK], f32, tag="mm")
                        for t in range(nt // 2):
                            nc.tensor.matmul(
                                ps,
                                v_sb[:, 2 * t:2 * t + 2, ht * P:(ht + 1) * P],
                                attnT[:, 2 * t:2 * t + 2, :],
                                start=(t == 0), stop=(t == nt // 2 - 1),
                                perf_mode=DR,
                            )
                        # 1/16 scale keeps the unnormalized sums inside
                        # fp8e4's +-240 range (max |outT| ~ 300 without it);
                        # compensated by the 16x in gr below.
                        nc.vector.tensor_scalar(
                            out=outT[:, ht, :], in0=ps, scalar1=0.0625,
                            scalar2=None, op0=OP.mult)

                    # column sums of attnT: ones.T @ attnT (DR), [1, QBLK]
                    cs = ps_cs.tile([1, QBLK], f32, tag="cs")
                    for t in range(nt // 2):
                        nc.tensor.matmul(
                            cs, ones8[:, :, 0:1],
                            attnT[:, 2 * t:2 * t + 2, :],
                            start=(t == 0), stop=(t == nt // 2 - 1),
                            perf_mode=DR,
                        )
                    csum = cspool.tile([1, QBLK], f32, tag="csum")
                    nc.vector.tensor_copy(out=csum, in_=cs)

                    # keep the PE dense across the batch boundary: next
                    # batch's kT matmuls run before the last epilogue,
                    # v right after it. qT is pipelined one block ahead.
                    # These matmuls also cover the csum-copy latency so the
                    # rank-1 scatter below starts without a PE stall.
                    if qb + 1 < nqb:
                        qstate = emit_qt(b, qb + 1, rgbT)
                    elif next_in is not None:
                        next_kT = emit_kT(next_in[0])
                        qstate = emit_qt(b + 1, 0, next_in[2])

                    # scatter csum across partitions: gr_all[p, j] = csum[j*128+p]
                    pst = ps_r1.tile([P, 4], f32, tag="r1")
                    for j in range(tpb):
                        nc.tensor.matmul(
                            pst[:, j:j + 1],
                            csum[0:1, j * P:(j + 1) * P], ones_1x4[:, 0:1],
                            start=True, stop=True,
                        )
                    rec = small.tile([P, tpb], f32, tag="rec")
                    nc.vector.reciprocal(out=rec, in_=pst[:, :tpb])
                    gr_all = small.tile([P, tpb], f32, tag="gr")
                    nc.vector.tensor_scalar_mul(
                        out=gr_all, in0=rec, scalar1=gate_sb)

                    # pass 1: proj matmuls, residual, LN stats (per row tile)
                    xs = []
                    mv_all = small.tile([P, 2 * tpb], f32, tag="mv")
                    for j in range(tpb):
                        psp = ps_mm.tile([P, d], f32, tag="mm")
                        for hh in range(nht // 2):
                            nc.tensor.matmul(
                                psp,
                                outT[:, 2 * hh:2 * hh + 2, j * P:(j + 1) * P],
                                wp_sb[:, 2 * hh:2 * hh + 2, :],
                                start=(hh == 0), stop=(hh == nht // 2 - 1),
                                perf_mode=DR,
                            )
                        # x = gr * proj + (rgb + gate*bp)
                        x = ypool.tile([P, d], f32, tag="x")
                        nc.vector.scalar_tensor_tensor(
                            out=x, in0=psp, scalar=gr_all[:, j:j + 1],
                            in1=rgb_raw[j],
                            op0=OP.mult, op1=OP.add,
                        )
                        stats = small.tile([P, 6], f32, tag="stats")
                        nc.vector.bn_stats(out=stats, in_=x)
                        nc.vector.bn_aggr(out=mv_all[:, 2 * j:2 * j + 2],
                                          in_=stats)
                        xs.append(x)

                    # pass 2: batched rsqrt over the block's row tiles, then
                    # normalize + affine + store
                    ve = small.tile([P, tpb], f32, tag="ve")
                    nc.vector.tensor_scalar(
                        out=ve, in0=mv_all[:, 1::2], scalar1=LN_EPS,
                        scalar2=None, op0=OP.add)
                    rstd = small.tile([P, tpb], f32, tag="rstd")
                    rsqrt_dve(ve, rstd, tpb)
                    for j in range(tpb):
                        x = xs[j]
                        nc.vector.tensor_scalar(
                            out=x, in0=x, scalar1=mv_all[:, 2 * j:2 * j + 1],
                            scalar2=rstd[:, j:j + 1],
                            op0=OP.subtract, op1=OP.mult,
                        )
                        nc.vector.tensor_mul(out=x, in0=x, in1=gamma_bc)
                        nc.vector.tensor_add(out=x, in0=x, in1=beta_bc)
                        nc.gpsimd.dma_start(
                            out=out[b, q0 + j * P:q0 + (j + 1) * P, :], in_=x)

                    if qb == nqb - 1 and next_in is not None:
                        next_v = emit_v(next_in[1])

                if next_in is not None:
                    kT, v_sb = next_kT, next_v

    nc.compile()
    return nc


_CACHE = {}


def kernel(**inputs):
    from concourse.bass_utils import run_bass_kernel_spmd

    if "nc" not in _CACHE:
        _CACHE["nc"] = build_nc()
    nc = _CACHE["nc"]

    staged = prepare_inputs(inputs)

    in_maps = []
    for i in range(N_CORES):
        m = {}
        for k, v in staged.items():
            if k in SHARDED_NAMES:
                m[k] = np.ascontiguousarray(v[i * B_LOC:(i + 1) * B_LOC])
            else:
                m[k] = v
        in_maps.append(m)

    res = run_bass_kernel_spmd(nc, in_maps, list(range(N_CORES))).results
    return np.concatenate([res[i]["out"] for i in range(N_CORES)], axis=0)


# revision 31
# speedup vs baseline: 1.1652x; 1.1652x over previous
"""CrossModalFusion Trainium2 kernel (v5).

Reference computation (per batch b):
    q = rgb @ Wq + bq                 [S, H]
    k = pose @ Wk + bk                [S, H]
    v = pose @ Wv + bv                [S, H]
    attn = softmax(q @ k.T / sqrt(H)) [S, S]
    out  = attn @ v                   [S, H]
    proj = out @ Wp + bp              [S, D]
    x = rgb + gate * proj
    fused = LayerNorm(x) * gamma + beta

Sharding: pure data-parallel over batch B=32 across 8 NeuronCores
(4 batches per core), identical SPMD program, no collectives.

Design notes:
  - Host pre-processing: rgb/pose cast to bf16, zero-padded d 400->512
    (full-128 contraction chunks keep the PE's HAM activity monitor at
    the 2.4 GHz un-throttled clock) and PRE-TRANSPOSED to
    [b, 4, 128, s] so the kernel only does contiguous DMA loads --
    zero transpose instructions on the device.
  - QKV biases are folded into the matmuls: padded input column 400
    is set to 1.0 and row 400 of the padded Wq/Wk/Wv carries the
    bias, so PSUM drains are plain dtype-converting copies.
  - QKV/proj matmuls in bf16, q/k/v/attn stored fp8e4; the two big
    attention matmuls + column sums run fp8 DoubleRow (256-deep
    contraction per instruction, ~2x bf16 FLOP rate).
  - exp computed as exp(s/sqrt(H) - SHIFT) so unnormalized weights fit
    fp8e4's +-240 range (TRN e4m3 NaNs above 240); the shift cancels
    in the softmax normalization.
  - ACT runs only Exp + PSUM copies (no activation-table reloads);
    LayerNorm rsqrt is a bit-trick+Newton chain on DVE, batched over
    each query block's four row tiles.
  - Next batch's kT matmuls are emitted before the last query block's
    epilogue (and v right after it) so the PE never idles across batch
    boundaries.
"""

import numpy as np

B, S, D, H = 32, 2048, 400, 512
DP = 512                 # d padded to a multiple of 128
N_CORES = 8
B_LOC = B // N_CORES
LN_EPS = 1e-5
P = 128                  # partitions
QBLK = 512               # query block (columns of scoresT)
SHIFT = 2.5              # exp(s - SHIFT): keeps fp8 attn weights < 240

WEIGHT_NAMES = ("Wq", "bq", "Wk", "bk", "Wv", "bv", "Wp", "bp",
                "ln_gamma", "ln_beta", "gate")
SHARDED_NAMES = ("poseT", "pose8T", "rgbT", "rgb32")


def prepare_inputs(inputs):
    """Host-side preprocessing: cast the big activations to bf16 (and
    fp8 for the v path), zero-pad d 400->DP with a 1.0 in column d
    (bias row), and pre-transpose to the [b, DP/128, 128, s] layout
    the kernel streams with plain contiguous DMAs. QKV biases ride in
    row d of the padded weights."""
    import ml_dtypes

    bf16 = ml_dtypes.bfloat16
    fp8 = ml_dtypes.float8_e4m3
    rgb = np.asarray(inputs["rgb"], dtype=np.float32)
    pose = np.asarray(inputs["pose"], dtype=np.float32)
    b, s, d = rgb.shape

    def pad_cast_t(x, dt):
        out = np.zeros((b, s, DP), dtype=dt)
        out[:, :, :d] = x.astype(dt)
        out[:, :, d] = 1.0          # bias row
        # [b, s, (c p)] -> [b, c, p, s]
        return np.ascontiguousarray(
            out.reshape(b, s, DP // P, P).transpose(0, 2, 3, 1))

    def pad_w(w, bias, dt):
        out = np.zeros((DP, H), dtype=dt)
        out[:d] = np.asarray(w, dtype=np.float32).astype(dt)
        out[d] = np.asarray(bias, dtype=np.float32).astype(dt)
        return out

    staged = {
        "poseT": pad_cast_t(pose, bf16),
        "pose8T": pad_cast_t(pose, fp8),
        "rgbT": pad_cast_t(rgb, bf16),
        "rgb32": np.ascontiguousarray(rgb),
        "Wq": pad_w(inputs["Wq"], inputs["bq"], bf16),
        "Wk": pad_w(inputs["Wk"], inputs["bk"], bf16),
        "Wv": pad_w(inputs["Wv"], inputs["bv"], fp8),
        "Wp": np.asarray(inputs["Wp"], dtype=np.float32).astype(bf16),
    }
    for name in ("bp", "ln_gamma", "ln_beta", "gate"):
        staged[name] = np.ascontiguousarray(inputs[name], dtype=np.float32)
    return staged


def build_nc(b_loc=B_LOC, s=S, d=D, h=H):
    import concourse.bass as bass
    import concourse.mybir as mybir
    import concourse.tile as tile
    from concourse import bacc

    def bcast(ap1d, p=P):
        """Broadcast a 1-D DRAM AP across p partitions (step-0 leading dim)."""
        return bass.AP(tensor=ap1d.tensor, offset=ap1d.offset,
                       ap=[[0, p]] + list(ap1d.ap))

    f32 = mybir.dt.float32
    bf16 = mybir.dt.bfloat16
    fp8 = mybir.dt.float8e4
    i32 = mybir.dt.int32
    AF = mybir.ActivationFunctionType
    DR = mybir.MatmulPerfMode.DoubleRow
    OP = mybir.AluOpType

    nt = s // P              # seq tiles (16)
    nqb = s // QBLK          # query blocks (4)
    tpb = QBLK // P          # seq tiles per query block (4)
    nht = h // P             # h chunks (4)
    ndc = DP // P            # padded-d chunks (4)
    scale = 1.0 / float(np.sqrt(h))

    nc = bacc.Bacc("TRN2", target_bir_lowering=False, debug=False,
                   num_swdge_queues=4)

    poseT_d = nc.dram_tensor("poseT", [b_loc, ndc, P, s], bf16,
                             kind="ExternalInput").ap()
    pose8T_d = nc.dram_tensor("pose8T", [b_loc, ndc, P, s], fp8,
                              kind="ExternalInput").ap()
    rgbT_d = nc.dram_tensor("rgbT", [b_loc, ndc, P, s], bf16,
                            kind="ExternalInput").ap()
    rgb32 = nc.dram_tensor("rgb32", [b_loc, s, d], f32,
                           kind="ExternalInput").ap()
    Wq = nc.dram_tensor("Wq", [DP, h], bf16, kind="ExternalInput").ap()
    Wk = nc.dram_tensor("Wk", [DP, h], bf16, kind="ExternalInput").ap()
    Wv = nc.dram_tensor("Wv", [DP, h], fp8, kind="ExternalInput").ap()
    Wp = nc.dram_tensor("Wp", [h, d], bf16, kind="ExternalInput").ap()
    bp = nc.dram_tensor("bp", [d], f32, kind="ExternalInput").ap()
    gamma = nc.dram_tensor("ln_gamma", [d], f32, kind="ExternalInput").ap()
    beta = nc.dram_tensor("ln_beta", [d], f32, kind="ExternalInput").ap()
    gate = nc.dram_tensor("gate", [1], f32, kind="ExternalInput").ap()
    out = nc.dram_tensor("out", [b_loc, s, d], f32, kind="ExternalOutput").ap()

    from contextlib import ExitStack

    with tile.TileContext(nc) as tc:
        with ExitStack() as ctx:
            pool = lambda **kw: ctx.enter_context(tc.tile_pool(**kw))
            const = pool(name="const", bufs=1)
            wpool = pool(name="wpool", bufs=1)
            ptp = pool(name="ptp", bufs=2)            # poseT bf16
            p8tp = pool(name="p8tp", bufs=2)          # poseT fp8
            rtp = pool(name="rtp", bufs=2)            # rgbT bf16
            ktp = pool(name="ktp", bufs=1)            # kT fp8
            vtp = pool(name="vtp", bufs=1)            # v fp8
            qtp = pool(name="qtp", bufs=2)            # qT fp8
            atp = pool(name="atp", bufs=2)            # attnT fp8
            otp = pool(name="otp", bufs=2)            # outT bf16
            rres = pool(name="rres", bufs=2 * tpb)    # rgb32 rows f32
            small = pool(name="small", bufs=6)
            cspool = pool(name="cspool", bufs=2)
            ypool = pool(name="ypool", bufs=2 * tpb)
            ps_mm = pool(name="ps_mm", bufs=4, space="PSUM")
            ps_sc = pool(name="ps_sc", bufs=2, space="PSUM")
            ps_cs = pool(name="ps_cs", bufs=1, space="PSUM")
            ps_r1 = pool(name="ps_r1", bufs=1, space="PSUM")

            # ---- constants ----
            ones8 = const.tile([P, 2, 16], fp8)
            nc.vector.memset(ones8, 1.0)
            ones_1x4 = const.tile([1, 4], f32)
            nc.vector.memset(ones_1x4, 1.0)
            nshift = const.tile([P, 1], f32)
            nc.vector.memset(nshift, -SHIFT)

            # input loads for batch 0 (pose chunks first: phase A only
            # needs poseT, so the PE can start as soon as they land)
            def emit_in_loads(b):
                poseT = ptp.tile([P, ndc, s], bf16, tag="poseT")
                pose8T = p8tp.tile([P, ndc, s], fp8, tag="pose8T")
                rgbT = rtp.tile([P, ndc, s], bf16, tag="rgbT")
                for c in range(ndc):
                    nc.sync.dma_start(out=poseT[:, c, :],
                                      in_=poseT_d[b, c, :, :])
                for c in range(ndc):
                    nc.sync.dma_start(out=pose8T[:, c, :],
                                      in_=pose8T_d[b, c, :, :])
                    nc.sync.dma_start(out=rgbT[:, c, :],
                                      in_=rgbT_d[b, c, :, :])
                return poseT, pose8T, rgbT

            in_state = emit_in_loads(0)

            # HAM warm-up: dense full-array matmuls bridging the initial
            # DMA window so phase A starts at the 2.4 GHz clock. Emitted
            # before the weight loads so the first matmul isn't queued
            # behind DMA-dependent DVE work.
            ident = const.tile([P, P], bf16)
            nc.vector.memset(ident, 0.0)
            warm = ps_mm.tile([P, QBLK], f32, tag="mm")
            for _ in range(140):
                nc.tensor.matmul(warm[:, :P], ident, ident,
                                 start=True, stop=True)

            # weights: Wk first (phase A starts with kT), then Wv, Wq, Wp
            wk_sb = wpool.tile([P, ndc, h], bf16)
            wv_sb = wpool.tile([P, ndc, h], fp8)
            wq_sb = wpool.tile([P, ndc, h], bf16)
            for dst, W in ((wk_sb, Wk), (wv_sb, Wv), (wq_sb, Wq)):
                for c in range(ndc):
                    nc.gpsimd.dma_start(out=dst[:, c, :],
                                        in_=W[c * P:(c + 1) * P, :])
            wp_sb = wpool.tile([P, nht, d], bf16)
            for t in range(nht):
                nc.gpsimd.dma_start(out=wp_sb[:, t, :],
                                    in_=Wp[t * P:(t + 1) * P, :])

            bp_bc = wpool.tile([P, d], f32)
            nc.gpsimd.dma_start(out=bp_bc, in_=bcast(bp))
            gamma_bc = wpool.tile([P, d], f32)
            nc.gpsimd.dma_start(out=gamma_bc, in_=bcast(gamma))
            beta_bc = wpool.tile([P, d], f32)
            nc.gpsimd.dma_start(out=beta_bc, in_=bcast(beta))
            gate_sb = wpool.tile([P, 1], f32)
            nc.gpsimd.dma_start(out=gate_sb, in_=bcast(gate))
            # bpg = gate * bp (added to rgb once per row tile)
            bpg_bc = wpool.tile([P, d], f32)
            nc.vector.tensor_scalar_mul(out=bpg_bc, in0=bp_bc, scalar1=gate_sb)

            def rsqrt_dve(var_eps, gbuf, n):
                """gbuf[:, :n] <- 1/sqrt(var_eps[:, :n]) on DVE.

                Bit-trick seed + 2 Newton iterations (final rel err <1e-5)."""
                gi = gbuf.bitcast(i32)
                nc.vector.tensor_scalar(
                    out=gi, in0=var_eps.bitcast(i32), scalar1=1, scalar2=None,
                    op0=OP.arith_shift_right)
                nc.vector.tensor_scalar(
                    out=gi, in0=gi, scalar1=-1, scalar2=0x5F3759DF,
                    op0=OP.mult, op1=OP.add)
                t = small.tile([P, n], f32, tag="nwt")
                for _ in range(2):
                    nc.vector.tensor_mul(out=t, in0=gbuf, in1=gbuf)
                    nc.vector.tensor_mul(out=t, in0=t, in1=var_eps)
                    nc.vector.tensor_scalar(
                        out=t, in0=t, scalar1=-0.5, scalar2=1.5,
                        op0=OP.mult, op1=OP.add)
                    nc.vector.tensor_mul(out=gbuf, in0=gbuf, in1=t)

            def emit_kT(poseT):
                """kT[h, :] = Wk.T-chunks @ poseT (bias folded in)."""
                kT = ktp.tile([P, nht, s], fp8, tag="kT")
                for ht in range(nht):
                    for nb in range(s // QBLK):
                        ps = ps_mm.tile([P, QBLK], f32, tag="mm")
                        for c in range(ndc):
                            nc.tensor.matmul(
                                ps,
                                wk_sb[:, c, ht * P:(ht + 1) * P],
                                poseT[:, c, nb * QBLK:(nb + 1) * QBLK],
                                start=(c == 0), stop=(c == ndc - 1),
                            )
                        nc.scalar.copy(
                            out=kT[:, ht, nb * QBLK:(nb + 1) * QBLK], in_=ps)
                return kT

            def emit_v(pose8T):
                """v[sk, h] seq-major (bias folded in), fp8 DoubleRow."""
                v_sb = vtp.tile([P, nt, h], fp8, tag="v")
                for t in range(nt):
                    ps = ps_mm.tile([P, h], f32, tag="mm")
                    for cc in range(ndc // 2):
                        nc.tensor.matmul(
                            ps,
                            pose8T[:, 2 * cc:2 * cc + 2, t * P:(t + 1) * P],
                            wv_sb[:, 2 * cc:2 * cc + 2, :],
                            start=(cc == 0), stop=(cc == ndc // 2 - 1),
                            perf_mode=DR,
                        )
                    nc.vector.tensor_copy(out=v_sb[:, t, :], in_=ps)
                return v_sb

            def emit_qt(b, qb, rgbT):
                """qT block (bf16 matmul, bias folded) + residual rows.

                Emitted one query block ahead so the qT drains clear the
                engine queues before scores need them."""
                q0 = qb * QBLK
                qT = qtp.tile([P, nht, QBLK], fp8, tag="qT")
                rgb_raw = []
                for j in range(tpb):
                    rr = rres.tile([P, d], f32, tag="rr")
                    nc.sync.dma_start(
                        out=rr, in_=rgb32[b, q0 + j * P:q0 + (j + 1) * P, :])
                    rgb_raw.append(rr)
                for ht in range(nht):
                    ps = ps_mm.tile([P, QBLK], f32, tag="mm")
                    for c in range(ndc):
                        nc.tensor.matmul(
                            ps,
                            wq_sb[:, c, ht * P:(ht + 1) * P],
                            rgbT[:, c, q0:q0 + QBLK],
                            start=(c == 0), stop=(c == ndc - 1),
                        )
                    nc.scalar.copy(out=qT[:, ht, :], in_=ps)
                # residual base: rgb += gate*bp
                for j in range(tpb):
                    nc.vector.tensor_add(
                        out=rgb_raw[j], in0=rgb_raw[j], in1=bpg_bc)
                return qT, rgb_raw

            kT, v_sb = emit_kT(in_state[0]), emit_v(in_state[1])
            qstate = emit_qt(0, 0, in_state[2])

            for b in range(b_loc):
                poseT, pose8T, rgbT = in_state
                next_in = next_kT = next_v = None
                if b + 1 < b_loc:
                    in_state = emit_in_loads(b + 1)
                    next_in = in_state

                for qb in range(nqb):
                    q0 = qb * QBLK
                    qT, rgb_raw = qstate

                    # scoresT tiles [sk 128, sq QBLK] via fp8 DoubleRow;
                    # exp(s*scale - SHIFT) -> attnT (fp8)
                    attnT = atp.tile([P, nt, QBLK], fp8, tag="attnT")
                    for c in range(nt):
                        ps = ps_sc.tile([P, QBLK], f32, tag="sc")
                        for jj in range(nht // 2):
                            nc.tensor.matmul(
                                ps,
                                kT[:, 2 * jj:2 * jj + 2, c * P:(c + 1) * P],
                                qT[:, 2 * jj:2 * jj + 2, :],
                                start=(jj == 0), stop=(jj == nht // 2 - 1),
                                perf_mode=DR,
                            )
                        nc.scalar.activation(
                            out=attnT[:, c, :], in_=ps, func=AF.Exp,
                            scale=scale, bias=nshift)

                    # outT[h, sq] = sum_t v[t].T-pair @ attnT[t-pair] (DR)
                    outT = otp.tile([P, nht, QBLK], bf16, tag="outT")
                    for ht in range(nht):
                        ps = ps_mm.tile([P, QBLK], f32, tag="mm")
                        for t in range(nt // 2):
                            nc.tensor.matmul(
                                ps,
                                v_sb[:, 2 * t:2 * t + 2, ht * P:(ht + 1) * P],
                                attnT[:, 2 * t:2 * t + 2, :],
                                start=(t == 0), stop=(t == nt // 2 - 1),
                                perf_mode=DR,
                            )
                        nc.vector.tensor_copy(out=outT[:, ht, :], in_=ps)

                    # column sums of attnT: ones.T @ attnT (DR), [1, QBLK]
                    cs = ps_cs.tile([1, QBLK], f32, tag="cs")
                    for t in range(nt // 2):
                        nc.tensor.matmul(
                            cs, ones8[:, :, 0:1],
                            attnT[:, 2 * t:2 * t + 2, :],
                            start=(t == 0), stop=(t == nt // 2 - 1),
                            perf_mode=DR,
                        )
                    csum = cspool.tile([1, QBLK], f32, tag="csum")
                    nc.vector.tensor_copy(out=csum, in_=cs)

                    # keep the PE dense across the batch boundary: next
                    # batch's kT matmuls run before the last epilogue,
                    # v right after it. qT is pipelined one block ahead.
                    # These matmuls also cover the csum-copy latency so the
                    # rank-1 scatter below starts without a PE stall.
                    if qb + 1 < nqb:
                        qstate = emit_qt(b, qb + 1, rgbT)
                    elif next_in is not None:
                        next_kT = emit_kT(next_in[0])
                        qstate = emit_qt(b + 1, 0, next_in[2])

                    # scatter csum across partitions: gr_all[p, j] = csum[j*128+p]
                    pst = ps_r1.tile([P, 4], f32, tag="r1")
                    for j in range(tpb):
                        nc.tensor.matmul(
                            pst[:, j:j + 1],
                            csum[0:1, j * P:(j + 1) * P], ones_1x4[:, 0:1],
                            start=True, stop=True,
                        )
                    rec = small.tile([P, tpb], f32, tag="rec")
                    nc.vector.reciprocal(out=rec, in_=pst[:, :tpb])
                    gr_all = small.tile([P, tpb], f32, tag="gr")
                    nc.vector.tensor_scalar_mul(
                        out=gr_all, in0=rec, scalar1=gate_sb)

                    # pass 1: proj matmuls, residual, LN stats (per row tile)
                    xs = []
                    mv_all = small.tile([P, 2 * tpb], f32, tag="mv")
                    for j in range(tpb):
                        psp = ps_mm.tile([P, d], f32, tag="mm")
                        for ht in range(nht):
                            nc.tensor.matmul(
                                psp,
                                outT[:, ht, j * P:(j + 1) * P],
                                wp_sb[:, ht, :],
                                start=(ht == 0), stop=(ht == nht - 1),
                            )
                        # x = gr * proj + (rgb + gate*bp)
                        x = ypool.tile([P, d], f32, tag="x")
                        nc.vector.scalar_tensor_tensor(
                            out=x, in0=psp, scalar=gr_all[:, j:j + 1],
                            in1=rgb_raw[j],
                            op0=OP.mult, op1=OP.add,
                        )
                        stats = small.tile([P, 6], f32, tag="stats")
                        nc.vector.bn_stats(out=stats, in_=x)
                        nc.vector.bn_aggr(out=mv_all[:, 2 * j:2 * j + 2],
                                          in_=stats)
                        xs.append(x)

                    # pass 2: batched rsqrt over the block's row tiles, then
                    # normalize + affine + store
                    ve = small.tile([P, tpb], f32, tag="ve")
                    nc.vector.tensor_scalar(
                        out=ve, in0=mv_all[:, 1::2], scalar1=LN_EPS,
                        scalar2=None, op0=OP.add)
                    rstd = small.tile([P, tpb], f32, tag="rstd")
                    rsqrt_dve(ve, rstd, tpb)
                    for j in range(tpb):
                        x = xs[j]
                        nc.vector.tensor_scalar(
                            out=x, in0=x, scalar1=mv_all[:, 2 * j:2 * j + 1],
                            scalar2=rstd[:, j:j + 1],
                            op0=OP.subtract, op1=OP.mult,
                        )
                        nc.vector.tensor_mul(out=x, in0=x, in1=gamma_bc)
                        nc.vector.tensor_add(out=x, in0=x, in1=beta_bc)
                        nc.gpsimd.dma_start(
                            out=out[b, q0 + j * P:q0 + (j + 1) * P, :], in_=x)

                    if qb == nqb - 1 and next_in is not None:
                        next_v = emit_v(next_in[1])

                if next_in is not None:
                    kT, v_sb = next_kT, next_v

    nc.compile()
    return nc


_CACHE = {}


def kernel(**inputs):
    from concourse.bass_utils import run_bass_kernel_spmd

    if "nc" not in _CACHE:
        _CACHE["nc"] = build_nc()
    nc = _CACHE["nc"]

    staged = prepare_inputs(inputs)

    in_maps = []
    for i in range(N_CORES):
        m = {}
        for k, v in staged.items():
            if k in SHARDED_NAMES:
                m[k] = np.ascontiguousarray(v[i * B_LOC:(i + 1) * B_LOC])
            else:
                m[k] = v
        in_maps.append(m)

    res = run_bass_kernel_spmd(nc, in_maps, list(range(N_CORES))).results
    return np.concatenate([res[i]["out"] for i in range(N_CORES)], axis=0)
